# revision 19
# baseline (speedup 1.0000x reference)
"""Trainium2 Bass kernel for nn_Attention_57243324121291.

Reference computation (shapes: L=2048, B=256, ENC_H=512, DEC_H=512, A=256):
    enc_q  = einsum('lbe,ae->bla', encoder_outputs, W_enc) + b_enc
    dec_q  = decoder_hidden @ W_dec.T + b_dec
    energy = tanh(einsum('bla,ba->bl', enc_q, dec_q))
    attn   = softmax(energy + encoder_mask, axis=1)[..., None]

Algebraic simplification (linearity of the contraction over a):
    energy[b,l] = tanh( sum_e enc[l,b,e] * v[b,e] + c[b] )
    with v = dec_q @ W_enc   [B, ENC_H]   (tiny -- computed host-side)
         c = dec_q @ b_enc   [B]
This avoids materializing the [B,L,A] intermediate entirely and turns the
kernel into a single streaming pass over encoder_outputs (memory-bound,
matching the target regime).

Sharding: data-parallel over B across 8 cores (32 batch rows per core).

Device strategy (per core):
  - encoder_outputs shard is pre-transposed on host to [b][e][l] fp16 and
    streamed as [128 e, 2048 l] tiles (contiguous per partition -> huge
    DMA descriptors, trivial descriptor-generation cost).
  - The e-contraction runs on the TensorEngine: for each (b, e-subtile)
    a masked stationary tile (zeros except column b = v[b] slice) is
    multiplied with the moving enc tile; all 128 matmuls accumulate into
    shared [32, 512] PSUM regions.  Rows m != b accumulate exact zeros,
    so PSUM ends up holding energy[b, l] directly in [b, l] layout -- no
    transpose/fold is ever needed.
  - Tail: ACT tanh(psum + c) -> SBUF, DVE mask add + softmax over the
    free dim, store [32, 2048] fp32.

fp16 ingestion halves HBM traffic (the kernel is DMA-bound); measured
output error vs the fp32 reference is ~7e-4 scale-relative absmax.
"""

import numpy as np

L, B, ENC_H, DEC_H, ATTN_H = 2048, 256, 512, 512, 256
N_CORES = 8
B_SH = B // N_CORES            # 32 batch rows per core
NSUB = ENC_H // 128            # 4 e-subtiles
NCH = L // 512                 # 4 l-chunks of 512
_PROG = None
_TRACE = False                 # test.py can flip this to collect a profile
_LAST_RESULTS = None           # test.py reads exec_time_ns etc. from here


def _legalize_waits(nc):
    """Move excess semaphore waits onto injected same-engine InstDrain carriers.

    The neuronx-cc codegen path allows very few sync-wait commands per
    instruction (custom DVE opcodes like TensorScalarPtr allow none, most
    compute instructions allow one).  Tile emits as many waits as the
    dependency structure needs, so instructions with several cross-engine
    dependencies fail codegen with "Too many sync wait commands".  Park
    the excess on chained single-wait InstDrain carriers.
    """
    import concourse.mybir as mybir

    for bb in nc.main_func.blocks:
        new_insts = []
        for ins in bb.instructions:
            si = ins.sync_info
            if si is not None and si.on_wait and not isinstance(
                    ins, mybir.InstEventSemaphore):
                allowed = 0 if isinstance(ins, mybir.InstTensorScalarPtr) else 1
                if len(si.on_wait) > allowed:
                    keep = si.on_wait[:allowed]
                    excess = si.on_wait[allowed:]
                    for w in excess:
                        new_insts.append(mybir.InstDrain(
                            name=nc.get_next_instruction_name(),
                            engine=ins.engine,
                            sync_info=mybir.SyncInfo(on_wait=[w],
                                                     on_update=[]),
                        ))
                    ins.sync_info = mybir.SyncInfo(
                        on_wait=list(keep), on_update=list(si.on_update))
            new_insts.append(ins)
        bb.instructions = new_insts


def _build_program():
    import concourse.bass as bass
    import concourse.mybir as mybir
    from concourse.tile import TileContext

    f32 = mybir.dt.float32
    f16 = mybir.dt.float16
    nc = bass.Bass()
    # enc: host-pre-transposed [(b, s, p), l] fp16; row (b*4+s)*128+p holds
    # encoder_outputs[l, b0+b, s*128+p] over l (contiguous per partition).
    enc = nc.declare_dram_parameter(
        "enc", [B_SH * ENC_H, L], f16, isOutput=False)
    # vmask: [128, (s*32+b)*32 + m] = v[b0+b, s*128+p] if m == b else 0
    vmask = nc.declare_dram_parameter(
        "vmask", [128, NSUB * B_SH * B_SH], f16, isOutput=False)
    cb = nc.declare_dram_parameter("cb", [B_SH, 1], f32, isOutput=False)
    mask = nc.declare_dram_parameter("mask", [B_SH, L], f32, isOutput=False)
    out = nc.declare_dram_parameter("out", [B_SH, L], f32, isOutput=True)

    with TileContext(nc) as tc:
        with tc.tile_pool(name="const", bufs=1) as cpool, \
             tc.tile_pool(name="io", bufs=5) as iopool, \
             tc.tile_pool(name="small", bufs=1) as spool, \
             tc.tile_pool(name="psum", bufs=1, space="PSUM") as pspool:
            vmt = cpool.tile([128, NSUB * B_SH * B_SH], f16)
            nc.sync.dma_start(out=vmt[:], in_=vmask[:, :])
            cbt = cpool.tile([B_SH, 1], f32)
            nc.sync.dma_start(out=cbt[:], in_=cb[:, :])
            maskt = spool.tile([B_SH, L], f32)
            nc.sync.dma_start(out=maskt[:], in_=mask[:, :])

            psum = [pspool.tile([B_SH, 512], f32, tag=f"ps{ch}",
                                name=f"psum{ch}")
                    for ch in range(NCH)]
            for b in range(B_SH):
                tile = iopool.tile([128, NSUB, L], f16, tag="enc")
                nc.sync.dma_start(
                    out=tile[:],
                    in_=enc[b * ENC_H:(b + 1) * ENC_H, :].rearrange(
                        "(s p) l -> p s l", p=128))
                for s in range(NSUB):
                    lhs = vmt[:, (s * B_SH + b) * B_SH:
                              (s * B_SH + b + 1) * B_SH]
                    first = (b == 0 and s == 0)
                    last = (b == B_SH - 1 and s == NSUB - 1)
                    for ch in range(NCH):
                        nc.tensor.matmul(
                            psum[ch][:, :], lhsT=lhs,
                            rhs=tile[:, s, ch * 512:(ch + 1) * 512],
                            start=first, stop=last)

            # tanh(energy + c) straight out of PSUM, per l-chunk
            et = spool.tile([B_SH, L], f32)
            for ch in range(NCH):
                nc.scalar.activation(
                    out=et[:, ch * 512:(ch + 1) * 512], in_=psum[ch][:, :],
                    func=mybir.ActivationFunctionType.Tanh, bias=cbt[:])

            # mask add + softmax over the free dim
            et2 = spool.tile([B_SH, L], f32)
            nc.vector.tensor_add(out=et2[:], in0=et[:], in1=maskt[:])
            nmax = spool.tile([B_SH, 1], f32)
            nc.vector.tensor_reduce(
                out=nmax[:], in_=et2[:], axis=mybir.AxisListType.X,
                op=mybir.AluOpType.max, negate=True)
            ex = spool.tile([B_SH, L], f32)
            sume = spool.tile([B_SH, 1], f32)
            nc.scalar.activation(
                out=ex[:], in_=et2[:], func=mybir.ActivationFunctionType.Exp,
                bias=nmax[:], accum_out=sume[:])
            rec = spool.tile([B_SH, 1], f32)
            nc.vector.reciprocal(out=rec[:], in_=sume[:])
            attn = spool.tile([B_SH, L], f32)
            nc.vector.tensor_scalar_mul(out=attn[:], in0=ex[:], scalar1=rec[:])
            nc.sync.dma_start(out=out[:, :], in_=attn[:])
    _legalize_waits(nc)
    return nc


def kernel(**inputs):
    global _PROG, _LAST_RESULTS
    enc = np.asarray(inputs["encoder_outputs"], dtype=np.float32)
    dh = np.asarray(inputs["decoder_hidden"], dtype=np.float32)
    msk = np.asarray(inputs["encoder_mask"], dtype=np.float32)
    W_enc = np.asarray(inputs["W_enc"], dtype=np.float32)
    b_enc = np.asarray(inputs["b_enc"], dtype=np.float32)
    W_dec = np.asarray(inputs["W_dec"], dtype=np.float32)
    b_dec = np.asarray(inputs["b_dec"], dtype=np.float32)

    dec_q = dh @ W_dec.T + b_dec          # [B, A]
    v = dec_q @ W_enc                     # [B, ENC_H]
    c = dec_q @ b_enc                     # [B]
    v16 = v.astype(np.float16)

    in_maps = []
    for i in range(N_CORES):
        b0 = i * B_SH
        # [l, b, e] -> [b, e, l] contiguous fp16
        enc_i = np.ascontiguousarray(
            enc[:, b0:b0 + B_SH, :].transpose(1, 2, 0)).astype(np.float16)
        enc_i = enc_i.reshape(B_SH * ENC_H, L)
        vm = np.zeros((128, NSUB, B_SH, B_SH), dtype=np.float16)
        for s in range(NSUB):
            for b in range(B_SH):
                vm[:, s, b, b] = v16[b0 + b, s * 128:(s + 1) * 128]
        vm = np.ascontiguousarray(vm.reshape(128, NSUB * B_SH * B_SH))
        cbi = np.ascontiguousarray(c[b0:b0 + B_SH][:, None])
        mi = np.ascontiguousarray(msk[b0:b0 + B_SH])
        in_maps.append({"enc": enc_i, "vmask": vm, "cb": cbi, "mask": mi})

    from concourse.bass_utils import run_bass_kernel_spmd
    if _PROG is None:
        _PROG = _build_program()
    res = run_bass_kernel_spmd(_PROG, in_maps, list(range(N_CORES)), trace=_TRACE)
    _LAST_RESULTS = res

    outs = [np.asarray(res.results[i]["out"]) for i in range(N_CORES)]
    return np.concatenate(outs, axis=0)[..., None].astype(np.float32)


# revision 21
# speedup vs baseline: 1.0681x; 1.0681x over previous
"""Trainium2 Bass kernel for nn_Attention_57243324121291.

Reference computation (shapes: L=2048, B=256, ENC_H=512, DEC_H=512, A=256):
    enc_q  = einsum('lbe,ae->bla', encoder_outputs, W_enc) + b_enc
    dec_q  = decoder_hidden @ W_dec.T + b_dec
    energy = tanh(einsum('bla,ba->bl', enc_q, dec_q))
    attn   = softmax(energy + encoder_mask, axis=1)[..., None]

Algebraic simplification (linearity of the contraction over a):
    energy[b,l] = tanh( sum_e enc[l,b,e] * v[b,e] + c[b] )
    with v = dec_q @ W_enc   [B, ENC_H]   (tiny -- computed host-side)
         c = dec_q @ b_enc   [B]
This avoids materializing the [B,L,A] intermediate entirely and turns the
kernel into a single streaming pass over encoder_outputs (memory-bound,
matching the target regime).

Sharding: data-parallel over B across 8 cores (32 batch rows per core).

Device strategy (per core):
  - encoder_outputs shard is pre-transposed on host to [b][e][l] fp16 and
    streamed as [128 e, 2048 l] tiles (contiguous per partition -> huge
    DMA descriptors, trivial descriptor-generation cost).
  - The e-contraction runs on the TensorEngine: for each (b, e-subtile)
    a masked stationary tile (zeros except column b = v[b] slice) is
    multiplied with the moving enc tile; all 128 matmuls accumulate into
    shared [32, 512] PSUM regions.  Rows m != b accumulate exact zeros,
    so PSUM ends up holding energy[b, l] directly in [b, l] layout -- no
    transpose/fold is ever needed.
  - Tail: ACT tanh(psum + c) -> SBUF, DVE mask add + softmax over the
    free dim, store [32, 2048] fp32.

fp16 ingestion halves HBM traffic (the kernel is DMA-bound); measured
output error vs the fp32 reference is ~7e-4 scale-relative absmax.
"""

import numpy as np

L, B, ENC_H, DEC_H, ATTN_H = 2048, 256, 512, 512, 256
N_CORES = 8
B_SH = B // N_CORES            # 32 batch rows per core
NSUB = ENC_H // 128            # 4 e-subtiles
NCH = L // 512                 # 4 l-chunks of 512
_PROG = None
_TRACE = False                 # test.py can flip this to collect a profile
_LAST_RESULTS = None           # test.py reads exec_time_ns etc. from here


def _legalize_waits(nc):
    """Move excess semaphore waits onto injected same-engine InstDrain carriers.

    The neuronx-cc codegen path allows very few sync-wait commands per
    instruction (custom DVE opcodes like TensorScalarPtr allow none, most
    compute instructions allow one).  Tile emits as many waits as the
    dependency structure needs, so instructions with several cross-engine
    dependencies fail codegen with "Too many sync wait commands".  Park
    the excess on chained single-wait InstDrain carriers.
    """
    import concourse.mybir as mybir

    for bb in nc.main_func.blocks:
        new_insts = []
        for ins in bb.instructions:
            si = ins.sync_info
            if si is not None and si.on_wait and not isinstance(
                    ins, mybir.InstEventSemaphore):
                allowed = 0 if isinstance(ins, mybir.InstTensorScalarPtr) else 1
                if len(si.on_wait) > allowed:
                    keep = si.on_wait[:allowed]
                    excess = si.on_wait[allowed:]
                    for w in excess:
                        new_insts.append(mybir.InstDrain(
                            name=nc.get_next_instruction_name(),
                            engine=ins.engine,
                            sync_info=mybir.SyncInfo(on_wait=[w],
                                                     on_update=[]),
                        ))
                    ins.sync_info = mybir.SyncInfo(
                        on_wait=list(keep), on_update=list(si.on_update))
            new_insts.append(ins)
        bb.instructions = new_insts


def _build_program():
    import concourse.bass as bass
    import concourse.mybir as mybir
    from concourse.tile import TileContext

    f32 = mybir.dt.float32
    f16 = mybir.dt.float16
    nc = bass.Bass()
    # enc: host-pre-transposed [(b, s, p), l] fp16; row (b*4+s)*128+p holds
    # encoder_outputs[l, b0+b, s*128+p] over l (contiguous per partition).
    enc = nc.declare_dram_parameter(
        "enc", [B_SH * ENC_H, L], f16, isOutput=False)
    # vmask: [128, (s*32+b)*32 + m] = v[b0+b, s*128+p] if m == b else 0
    vmask = nc.declare_dram_parameter(
        "vmask", [128, NSUB * B_SH * B_SH], f16, isOutput=False)
    cb = nc.declare_dram_parameter("cb", [B_SH, 1], f32, isOutput=False)
    mask = nc.declare_dram_parameter("mask", [B_SH, L], f32, isOutput=False)
    out = nc.declare_dram_parameter("out", [B_SH, L], f32, isOutput=True)

    with TileContext(nc) as tc:
        with tc.tile_pool(name="const", bufs=1) as cpool, \
             tc.tile_pool(name="io", bufs=16) as iopool, \
             tc.tile_pool(name="small", bufs=1) as spool, \
             tc.tile_pool(name="psum", bufs=1, space="PSUM") as pspool:
            vmt = cpool.tile([128, NSUB * B_SH * B_SH], f16)
            nc.sync.dma_start(out=vmt[:], in_=vmask[:, :])
            cbt = cpool.tile([B_SH, 1], f32)
            nc.sync.dma_start(out=cbt[:], in_=cb[:, :])
            maskt = spool.tile([B_SH, L], f32)
            nc.sync.dma_start(out=maskt[:], in_=mask[:, :])

            psum = [pspool.tile([B_SH, 512], f32, tag=f"ps{ch}",
                                name=f"psum{ch}")
                    for ch in range(NCH)]
            for b in range(B_SH):
                for s in range(NSUB):
                    tile = iopool.tile([128, L], f16, tag="enc")
                    r0 = (b * NSUB + s) * 128
                    nc.sync.dma_start(out=tile[:], in_=enc[r0:r0 + 128, :])
                    lhs = vmt[:, (s * B_SH + b) * B_SH:
                              (s * B_SH + b + 1) * B_SH]
                    first = (b == 0 and s == 0)
                    last = (b == B_SH - 1 and s == NSUB - 1)
                    for ch in range(NCH):
                        nc.tensor.matmul(
                            psum[ch][:, :], lhsT=lhs,
                            rhs=tile[:, ch * 512:(ch + 1) * 512],
                            start=first, stop=last)

            # tanh(energy + c) straight out of PSUM, per l-chunk
            et = spool.tile([B_SH, L], f32)
            for ch in range(NCH):
                nc.scalar.activation(
                    out=et[:, ch * 512:(ch + 1) * 512], in_=psum[ch][:, :],
                    func=mybir.ActivationFunctionType.Tanh, bias=cbt[:])

            # mask add + softmax over the free dim
            et2 = spool.tile([B_SH, L], f32)
            nc.vector.tensor_add(out=et2[:], in0=et[:], in1=maskt[:])
            nmax = spool.tile([B_SH, 1], f32)
            nc.vector.tensor_reduce(
                out=nmax[:], in_=et2[:], axis=mybir.AxisListType.X,
                op=mybir.AluOpType.max, negate=True)
            ex = spool.tile([B_SH, L], f32)
            sume = spool.tile([B_SH, 1], f32)
            nc.scalar.activation(
                out=ex[:], in_=et2[:], func=mybir.ActivationFunctionType.Exp,
                bias=nmax[:], accum_out=sume[:])
            rec = spool.tile([B_SH, 1], f32)
            nc.vector.reciprocal(out=rec[:], in_=sume[:])
            attn = spool.tile([B_SH, L], f32)
            nc.vector.tensor_scalar_mul(out=attn[:], in0=ex[:], scalar1=rec[:])
            nc.sync.dma_start(out=out[:, :], in_=attn[:])
    _legalize_waits(nc)
    return nc


def kernel(**inputs):
    global _PROG, _LAST_RESULTS
    enc = np.asarray(inputs["encoder_outputs"], dtype=np.float32)
    dh = np.asarray(inputs["decoder_hidden"], dtype=np.float32)
    msk = np.asarray(inputs["encoder_mask"], dtype=np.float32)
    W_enc = np.asarray(inputs["W_enc"], dtype=np.float32)
    b_enc = np.asarray(inputs["b_enc"], dtype=np.float32)
    W_dec = np.asarray(inputs["W_dec"], dtype=np.float32)
    b_dec = np.asarray(inputs["b_dec"], dtype=np.float32)

    dec_q = dh @ W_dec.T + b_dec          # [B, A]
    v = dec_q @ W_enc                     # [B, ENC_H]
    c = dec_q @ b_enc                     # [B]
    v16 = v.astype(np.float16)

    in_maps = []
    for i in range(N_CORES):
        b0 = i * B_SH
        # [l, b, e] -> [b, e, l] contiguous fp16
        enc_i = np.ascontiguousarray(
            enc[:, b0:b0 + B_SH, :].transpose(1, 2, 0)).astype(np.float16)
        enc_i = enc_i.reshape(B_SH * ENC_H, L)
        vm = np.zeros((128, NSUB, B_SH, B_SH), dtype=np.float16)
        for s in range(NSUB):
            for b in range(B_SH):
                vm[:, s, b, b] = v16[b0 + b, s * 128:(s + 1) * 128]
        vm = np.ascontiguousarray(vm.reshape(128, NSUB * B_SH * B_SH))
        cbi = np.ascontiguousarray(c[b0:b0 + B_SH][:, None])
        mi = np.ascontiguousarray(msk[b0:b0 + B_SH])
        in_maps.append({"enc": enc_i, "vmask": vm, "cb": cbi, "mask": mi})

    from concourse.bass_utils import run_bass_kernel_spmd
    if _PROG is None:
        _PROG = _build_program()
    res = run_bass_kernel_spmd(_PROG, in_maps, list(range(N_CORES)), trace=_TRACE)
    _LAST_RESULTS = res

    outs = [np.asarray(res.results[i]["out"]) for i in range(N_CORES)]
    return np.concatenate(outs, axis=0)[..., None].astype(np.float32)
